# revision 3
# baseline (speedup 1.0000x reference)
"""Trainium2 Bass kernel for nn_MixtureAttention (B=2, S=2048, D=1024, H=16).

Sharding: 8 cores = 2 batches x 4 head-groups (4 heads each, Megatron-style
tensor parallel). Each core computes, for its batch b and feature slice
fsl (256 features = 4 heads):

  x_pe   = x[b] + mixed_pe                       (pe terms folded on host)
  Q^T    = (Wq[fsl] x_pe^T) + (pe Wq^T + bq)^T   [256, 2048]  (f32r matmuls)
  K^T    likewise
  V      = x_pe Wv[fsl]^T + ...                  [2048, 256] natural layout
  S^T    = K_h^T.T-style scores per head:        [k_tok, q] tiles in PSUM
  P^T    = exp(S^T / 32)                         (no max-sub; scores are O(1))
  A^T    = (V_aug^T P^T) with ones column -> row 64 = softmax denominators
  apn    = A^T rows / denom row                  (normalized, [f, t] layout)
  y^T   += Wo[:, fsl]^T.T apn                    partial out-proj [1024, 2048]

Host sums the 4 partial y^T per batch, transposes, adds bo.
"""

import sys

sys.path.insert(0, "/opt/trn_rl_repo")

import numpy as np

import concourse.bass as bass
import concourse.mybir as mybir
import concourse.tile as tile
from concourse import bacc
from concourse.bass_utils import run_bass_kernel_spmd

F32 = mybir.dt.float32
F32R = mybir.dt.float32r
AF = mybir.ActivationFunctionType
ALU = mybir.AluOpType

B, S, D, H = 2, 2048, 1024, 16
MAX_SEQ_LEN = 5000
NCORES = 8
F = D // 4  # 256 features (4 heads) per core
HD = D // H  # 64
DT = D // 128  # 8 d-tiles
TT = S // 128  # 16 token tiles
QC = S // 512  # 4 q-chunks of 512
SCALE = 1.0 / np.sqrt(np.float32(D))  # 1/32


def build_nc():
    nc = bacc.Bacc("TRN2", target_bir_lowering=False, debug=False, num_devices=NCORES)

    xt_d = nc.declare_dram_parameter("xt", [D, S], F32R, isOutput=False)
    wqt_d = nc.declare_dram_parameter("wqt", [D, F], F32R, isOutput=False)
    wkt_d = nc.declare_dram_parameter("wkt", [D, F], F32R, isOutput=False)
    wvt_d = nc.declare_dram_parameter("wvt", [D, F], F32R, isOutput=False)
    wot_d = nc.declare_dram_parameter("wot", [F, D], F32R, isOutput=False)
    pq_d = nc.declare_dram_parameter("pq", [F, S], F32, isOutput=False)
    pk_d = nc.declare_dram_parameter("pk", [F, S], F32, isOutput=False)
    pv_d = nc.declare_dram_parameter("pv", [S, F], F32, isOutput=False)
    yt_d = nc.declare_dram_parameter("yt", [D, S], F32, isOutput=True)

    with tile.TileContext(nc) as tc:
        with (
            tc.tile_pool(name="persist", bufs=1) as pp,
            tc.tile_pool(name="xin", bufs=1) as xp,
            tc.tile_pool(name="pe_in", bufs=2) as pep,
            tc.tile_pool(name="expp", bufs=3) as expp,
            tc.tile_pool(name="pvs", bufs=2) as pvsp,
            tc.tile_pool(name="rcs", bufs=2) as rcp,
            tc.tile_pool(name="stage", bufs=2) as stp,
            tc.tile_pool(name="yst", bufs=2) as ystp,
            tc.tile_pool(name="ps_sc", bufs=2, space="PSUM") as ps_sc,
            tc.tile_pool(name="ps_mm", bufs=4, space="PSUM") as ps_mm,
        ):
            # ---- persistent SBUF ----
            wq = pp.tile([128, DT, F], F32R)
            wk = pp.tile([128, DT, F], F32R)
            wv = pp.tile([128, DT, F], F32R)
            wo = pp.tile([128, 2, D], F32R)
            qt = pp.tile([128, 2, S], F32R)  # Q^T, f-tile major
            kt = pp.tile([128, 2, S], F32R)
            vaug = pp.tile([128, TT, 4, HD + 1], F32R)  # [t, tt, head, hd+1]
            apn = pp.tile([128, 2, S], F32R)  # normalized A^T
            ones_bc = pp.tile([128, HD], F32R)

            nc.sync.dma_start(wq[:], wqt_d.rearrange("(dt p) f -> p dt f", p=128))
            nc.sync.dma_start(wk[:], wkt_d.rearrange("(dt p) f -> p dt f", p=128))
            nc.sync.dma_start(wv[:], wvt_d.rearrange("(dt p) f -> p dt f", p=128))
            nc.sync.dma_start(wo[:], wot_d.rearrange("(ft p) m -> p ft m", p=128))

            xt = xp.tile([128, DT, S], F32R)
            for dt in range(DT):
                nc.sync.dma_start(xt[:, dt], xt_d[dt * 128 : (dt + 1) * 128, :])

            ones_f32 = pp.tile([128, HD], F32)
            nc.vector.memset(ones_f32[:], 1.0)
            nc.vector.tensor_copy(ones_bc[:], ones_f32[:])
            # fill the per-head ones columns of V_aug (softmax denominators)
            nc.vector.tensor_copy(
                vaug[:, :, :, HD : HD + 1],
                ones_f32.rearrange("p (t h o) -> p t h o", t=TT, h=4),
            )

            # ---- V = x_pe @ Wv^T (+ pe/bias term), natural [t, f] layout ----
            for tt in range(TT):
                pvc = pep.tile([128, F], F32, tag="pvc")
                nc.sync.dma_start(pvc[:], pv_d[tt * 128 : (tt + 1) * 128, :])
                ps = ps_mm.tile([128, 512], F32, tag="mm")
                for dt in range(DT):
                    nc.tensor.matmul(
                        ps[:, :F],
                        xt[:, dt, tt * 128 : (tt + 1) * 128],
                        wv[:, dt],
                        start=(dt == 0),
                        stop=(dt == DT - 1),
                    )
                nc.vector.tensor_tensor(
                    vaug[:, tt, :, 0:HD],
                    ps[:, :F].rearrange("p (h e) -> p h e", h=4),
                    pvc.rearrange("p (h e) -> p h e", h=4),
                    ALU.add,
                )

            def qkt_chunk(dst, w, pe_d, ft, qcc, tag):
                """One [128, 512] chunk of Q^T or K^T (f-tile ft, q-chunk qcc)."""
                pec = pep.tile([128, 512], F32, tag=tag)
                nc.sync.dma_start(
                    pec[:],
                    pe_d[ft * 128 : (ft + 1) * 128, qcc * 512 : (qcc + 1) * 512],
                )
                ps = ps_mm.tile([128, 512], F32, tag="mm")
                for dt in range(DT):
                    nc.tensor.matmul(
                        ps[:],
                        w[:, dt, ft * 128 : (ft + 1) * 128],
                        xt[:, dt, qcc * 512 : (qcc + 1) * 512],
                        start=(dt == 0),
                        stop=(dt == DT - 1),
                    )
                nc.vector.tensor_tensor(
                    dst[:, ft, qcc * 512 : (qcc + 1) * 512], ps[:], pec[:], ALU.add
                )

            for qcc in range(QC):
                qkt_chunk(qt, wq, pq_d, 0, qcc, "pqc")
            for qcc in range(QC):
                qkt_chunk(kt, wk, pk_d, 0, qcc, "pkc")

            # ft=1 chunk generator, interleaved into pair-0 attention below
            ft1_work = []
            for qcc in range(QC):
                ft1_work.append((qt, wq, pq_d, "pqc"))
            for qcc in range(QC):
                ft1_work.append((kt, wk, pk_d, "pkc"))
            ft1_iter = iter(
                [(dst, w, ped, 1, i % QC, tag) for i, (dst, w, ped, tag) in enumerate(ft1_work)]
            )

            # ---- attention per head pair p (heads 2p at part 0:64, 2p+1 at 64:128) ----
            def attention_pair(p, interleave):
                step = 0
                for qc in range(QC):
                    pv0 = ps_mm.tile([128, 512], F32, tag="mm")
                    pv1 = ps_mm.tile([128, 512], F32, tag="mm")
                    for k in range(TT):
                        sc = ps_sc.tile([128, 1024], F32, tag="sc")
                        nc.tensor.matmul(
                            sc[:, 0:512],
                            kt[0:64, p, k * 128 : (k + 1) * 128],
                            qt[0:64, p, qc * 512 : (qc + 1) * 512],
                            start=True,
                            stop=True,
                        )
                        nc.tensor.matmul(
                            sc[:, 512:1024],
                            kt[64:128, p, k * 128 : (k + 1) * 128],
                            qt[64:128, p, qc * 512 : (qc + 1) * 512],
                            start=True,
                            stop=True,
                        )
                        ex = expp.tile([128, 1024], F32R, tag="ex")
                        nc.scalar.activation(ex[:], sc[:], AF.Exp, scale=float(SCALE))
                        nc.tensor.matmul(
                            pv0[0:65, :],
                            vaug[:, k, 2 * p, :],
                            ex[:, 0:512],
                            start=(k == 0),
                            stop=(k == TT - 1),
                        )
                        nc.tensor.matmul(
                            pv1[0:65, :],
                            vaug[:, k, 2 * p + 1, :],
                            ex[:, 512:1024],
                            start=(k == 0),
                            stop=(k == TT - 1),
                        )
                        if interleave and step % 8 == 7:
                            try:
                                args = next(ft1_iter)
                                qkt_chunk(*args)
                            except StopIteration:
                                pass
                        step += 1
                    # normalize: rows 0:64 / row 64 (denominator)
                    for h, pv in ((2 * p, pv0), (2 * p + 1, pv1)):
                        pvs = pvsp.tile([65, 512], F32, tag="pvs")
                        nc.vector.tensor_copy(pvs[:], pv[0:65, :])
                        rc = rcp.tile([65, 512], F32R, tag="rc")
                        with nc.allow_low_precision(reason="f32r for matmul rhs"):
                            nc.vector.reciprocal(rc[64:65, :], pvs[64:65, :])
                        bc = ps_sc.tile([128, 1024], F32, tag="sc")
                        nc.tensor.matmul(
                            bc[0:64, 0:512],
                            ones_bc[64:65, :],
                            rc[64:65, :],
                            start=True,
                            stop=True,
                        )
                        if h % 2 == 0:
                            nc.vector.tensor_tensor(
                                apn[0:64, p, qc * 512 : (qc + 1) * 512],
                                pvs[0:64, :],
                                bc[0:64, 0:512],
                                ALU.mult,
                            )
                        else:
                            st = stp.tile([64, 512], F32R, tag="st")
                            nc.vector.tensor_tensor(
                                st[:], pvs[0:64, :], bc[0:64, 0:512], ALU.mult
                            )
                            nc.sync.dma_start(
                                apn[64:128, p, qc * 512 : (qc + 1) * 512], st[:]
                            )

            attention_pair(0, interleave=True)
            # any leftover ft=1 chunks (shouldn't be, 8 groups / 8 slots)
            for args in ft1_iter:
                qkt_chunk(*args)
            attention_pair(1, interleave=False)

            # ---- out-projection: y^T[m, t] = sum_ft WoT[ft].T @ apn[ft] ----
            for mt in range(DT):
                for tcc in range(QC):
                    ps = ps_mm.tile([128, 512], F32, tag="mm")
                    for ft in range(2):
                        nc.tensor.matmul(
                            ps[:],
                            wo[:, ft, mt * 128 : (mt + 1) * 128],
                            apn[:, ft, tcc * 512 : (tcc + 1) * 512],
                            start=(ft == 0),
                            stop=(ft == 1),
                        )
                    yst = ystp.tile([128, 512], F32, tag="yst")
                    nc.scalar.activation(yst[:], ps[:], AF.Copy, bias=0.0)
                    nc.sync.dma_start(
                        yt_d[mt * 128 : (mt + 1) * 128, tcc * 512 : (tcc + 1) * 512],
                        yst[:],
                    )

    nc.compile()
    return nc


_NC = None


def _get_nc():
    global _NC
    if _NC is None:
        _NC = build_nc()
    return _NC


def _sinusoid_pe():
    pos = np.arange(MAX_SEQ_LEN, dtype=np.float32)[:, None]
    div = np.exp(
        np.arange(0, D, 2, dtype=np.float32) * np.float32(-np.log(10000.0) / D)
    )
    ang = pos * div[None, :]
    pe = np.stack([np.sin(ang), np.cos(ang)], axis=-1).reshape(MAX_SEQ_LEN, D)
    return pe.astype(np.float32)


def make_in_maps(x, rel_emb, alpha, Wq, bq, Wk, bk, Wv, bv, Wo, bo):
    alpha = np.float32(alpha)
    abs_pe = _sinusoid_pe()[:S]
    rel_pe = rel_emb[MAX_SEQ_LEN - S : MAX_SEQ_LEN]
    pe = (alpha * abs_pe + (np.float32(1.0) - alpha) * rel_pe).astype(np.float32)

    in_maps = []
    for c in range(NCORES):
        b, g = divmod(c, 4)
        fsl = slice(g * F, (g + 1) * F)
        wq_s, wk_s, wv_s = Wq[fsl], Wk[fsl], Wv[fsl]
        in_maps.append(
            {
                "xt": np.ascontiguousarray(x[b].T),
                "wqt": np.ascontiguousarray(wq_s.T),
                "wkt": np.ascontiguousarray(wk_s.T),
                "wvt": np.ascontiguousarray(wv_s.T),
                "wot": np.ascontiguousarray(Wo[:, fsl].T),
                "pq": np.ascontiguousarray((pe @ wq_s.T + bq[fsl]).T),
                "pk": np.ascontiguousarray((pe @ wk_s.T + bk[fsl]).T),
                "pv": np.ascontiguousarray(pe @ wv_s.T + bv[fsl]),
            }
        )
    return in_maps


def unshard(results, bo):
    y = np.empty((B, S, D), dtype=np.float32)
    for b in range(B):
        acc = results[4 * b]["yt"].astype(np.float32).copy()
        for g in range(1, 4):
            acc += results[4 * b + g]["yt"]
        y[b] = acc.T + bo
    return y


def kernel(x, rel_emb, alpha, Wq, bq, Wk, bk, Wv, bv, Wo, bo, **kw):
    x = np.asarray(x, dtype=np.float32)
    args = [np.asarray(a, dtype=np.float32) for a in (rel_emb, alpha, Wq, bq, Wk, bk, Wv, bv, Wo, bo)]
    nc = _get_nc()
    in_maps = make_in_maps(x, *args)
    res = run_bass_kernel_spmd(nc, in_maps, core_ids=list(range(NCORES)))
    return unshard(res.results, args[-1])


# revision 4
# speedup vs baseline: 56.7708x; 56.7708x over previous
"""Trainium2 Bass kernel for nn_MixtureAttention (B=2, S=2048, D=1024, H=16).

Sharding: 8 cores = 2 batches x 4 head-groups (4 heads each, Megatron-style
tensor parallel). Each core computes, for its batch b and feature slice
fsl (256 features = 4 heads):

  x_pe   = x[b] + mixed_pe                       (pe terms folded on host)
  Q^T    = (Wq[fsl] x_pe^T) + (pe Wq^T + bq)^T   [256, 2048]  (f32r matmuls)
  K^T    likewise
  V      = x_pe Wv[fsl]^T + ...                  [2048, 256] natural layout
  S^T    = scores per head in [k_tok, q] tiles (PSUM)
  P^T    = exp(S^T / 32)                         (no max-sub; scores are O(1))
  A^T    = V_aug^T-style matmul with ones column -> row 64 = softmax denoms
  apn    = A^T rows / denom row                  (normalized, [f, t] layout)
  y^T   += Wo[:, fsl]^T.T apn                    partial out-proj [1024, 2048]

Host sums the 4 partial y^T per batch, transposes, adds bo.

build_nc(iters=N) repeats the whole computation N times inside one NEFF so
marginal per-iteration time can be measured through the (large, constant)
axon dispatch overhead.
"""

import sys

sys.path.insert(0, "/opt/trn_rl_repo")

import numpy as np

import concourse.bass as bass
import concourse.mybir as mybir
import concourse.tile as tile
from concourse import bacc
from concourse.bass_utils import run_bass_kernel_spmd

F32 = mybir.dt.float32
F32R = mybir.dt.float32r
AF = mybir.ActivationFunctionType
ALU = mybir.AluOpType

B, S, D, H = 2, 2048, 1024, 16
MAX_SEQ_LEN = 5000
NCORES = 8
F = D // 4  # 256 features (4 heads) per core
HD = D // H  # 64
DT = D // 128  # 8 d-tiles
TT = S // 128  # 16 token tiles
QC = S // 512  # 4 q-chunks of 512
SCALE = 1.0 / np.sqrt(np.float32(D))  # 1/32


def build_nc(iters=1):
    nc = bacc.Bacc("TRN2", target_bir_lowering=False, debug=False, num_devices=NCORES)

    xt_d = nc.declare_dram_parameter("xt", [D, S], F32R, isOutput=False)
    wqt_d = nc.declare_dram_parameter("wqt", [D, F], F32R, isOutput=False)
    wkt_d = nc.declare_dram_parameter("wkt", [D, F], F32R, isOutput=False)
    wvt_d = nc.declare_dram_parameter("wvt", [D, F], F32R, isOutput=False)
    wot_d = nc.declare_dram_parameter("wot", [F, D], F32R, isOutput=False)
    pq_d = nc.declare_dram_parameter("pq", [F, S], F32, isOutput=False)
    pk_d = nc.declare_dram_parameter("pk", [F, S], F32, isOutput=False)
    pv_d = nc.declare_dram_parameter("pv", [S, F], F32, isOutput=False)
    yt_d = nc.declare_dram_parameter("yt", [D, S], F32, isOutput=True)

    with tile.TileContext(nc) as tc:
        with (
            tc.tile_pool(name="persist", bufs=1) as pp,
            tc.tile_pool(name="xin", bufs=1) as xp,
            tc.tile_pool(name="pe_in", bufs=2) as pep,
            tc.tile_pool(name="expp", bufs=3) as expp,
            tc.tile_pool(name="pvs", bufs=2) as pvsp,
            tc.tile_pool(name="rcs", bufs=2) as rcp,
            tc.tile_pool(name="stage", bufs=2) as stp,
            tc.tile_pool(name="yst", bufs=2) as ystp,
            tc.tile_pool(name="ps_sc", bufs=2, space="PSUM") as ps_sc,
            tc.tile_pool(name="ps_mm", bufs=4, space="PSUM") as ps_mm,
        ):
            # ---- persistent SBUF ----
            wq = pp.tile([128, DT, F], F32R)
            wk = pp.tile([128, DT, F], F32R)
            wv = pp.tile([128, DT, F], F32R)
            wo = pp.tile([128, 2, D], F32R)
            qt = pp.tile([128, 2, S], F32R)  # Q^T, f-tile major
            kt = pp.tile([128, 2, S], F32R)
            vaug = pp.tile([128, TT, 4, HD + 1], F32R)  # [t, tt, head, hd+1]
            apn = pp.tile([128, 2, S], F32R)  # normalized A^T
            ones_bc = pp.tile([128, HD], F32R)
            ones_f32 = pp.tile([128, HD], F32)

            nc.sync.dma_start(wq[:], wqt_d.rearrange("(dt p) f -> p dt f", p=128))
            nc.sync.dma_start(wk[:], wkt_d.rearrange("(dt p) f -> p dt f", p=128))
            nc.sync.dma_start(wv[:], wvt_d.rearrange("(dt p) f -> p dt f", p=128))
            nc.sync.dma_start(wo[:], wot_d.rearrange("(ft p) m -> p ft m", p=128))

            nc.vector.memset(ones_f32[:], 1.0)
            nc.vector.tensor_copy(ones_bc[:], ones_f32[:])
            # fill the per-head ones columns of V_aug (softmax denominators)
            nc.vector.tensor_copy(
                vaug[:, :, :, HD : HD + 1],
                ones_f32.rearrange("p (t h o) -> p t h o", t=TT, h=4),
            )

            for _it in range(iters):
                body(nc, tc, locals())

    nc.compile()
    return nc


def body(nc, tc, env):
    """Emit one full forward pass (called `iters` times)."""
    pp = env["pp"]
    xp = env["xp"]
    pep = env["pep"]
    expp = env["expp"]
    pvsp = env["pvsp"]
    rcp = env["rcp"]
    stp = env["stp"]
    ystp = env["ystp"]
    ps_sc = env["ps_sc"]
    ps_mm = env["ps_mm"]
    wq, wk, wv, wo = env["wq"], env["wk"], env["wv"], env["wo"]
    qt, kt, vaug, apn = env["qt"], env["kt"], env["vaug"], env["apn"]
    ones_bc = env["ones_bc"]
    xt_d, pq_d, pk_d, pv_d, yt_d = (
        env["xt_d"], env["pq_d"], env["pk_d"], env["pv_d"], env["yt_d"],
    )

    xt = xp.tile([128, DT, S], F32R, tag="xt")
    for dt in range(DT):
        nc.sync.dma_start(xt[:, dt], xt_d[dt * 128 : (dt + 1) * 128, :])

    # ---- V = x_pe @ Wv^T (+ pe/bias term), natural [t, f] layout ----
    for tt in range(TT):
        pvc = pep.tile([128, F], F32, tag="pvc")
        nc.sync.dma_start(pvc[:], pv_d[tt * 128 : (tt + 1) * 128, :])
        ps = ps_mm.tile([128, 512], F32, tag="mm")
        for dt in range(DT):
            nc.tensor.matmul(
                ps[:, :F],
                xt[:, dt, tt * 128 : (tt + 1) * 128],
                wv[:, dt],
                start=(dt == 0),
                stop=(dt == DT - 1),
            )
        nc.vector.tensor_tensor(
            vaug[:, tt, :, 0:HD],
            ps[:, :F].rearrange("p (h e) -> p h e", h=4),
            pvc.rearrange("p (h e) -> p h e", h=4),
            ALU.add,
        )

    def qkt_chunk(dst, w, pe_d, ft, qcc, tag):
        """One [128, 512] chunk of Q^T or K^T (f-tile ft, q-chunk qcc)."""
        pec = pep.tile([128, 512], F32, tag=tag)
        nc.sync.dma_start(
            pec[:],
            pe_d[ft * 128 : (ft + 1) * 128, qcc * 512 : (qcc + 1) * 512],
        )
        ps = ps_mm.tile([128, 512], F32, tag="mm")
        for dt in range(DT):
            nc.tensor.matmul(
                ps[:],
                w[:, dt, ft * 128 : (ft + 1) * 128],
                xt[:, dt, qcc * 512 : (qcc + 1) * 512],
                start=(dt == 0),
                stop=(dt == DT - 1),
            )
        nc.vector.tensor_tensor(
            dst[:, ft, qcc * 512 : (qcc + 1) * 512], ps[:], pec[:], ALU.add
        )

    for qcc in range(QC):
        qkt_chunk(qt, wq, pq_d, 0, qcc, "pqc")
    for qcc in range(QC):
        qkt_chunk(kt, wk, pk_d, 0, qcc, "pkc")

    # ft=1 chunks get interleaved into pair-0 attention below
    ft1_list = [(qt, wq, pq_d, 1, i, "pqc") for i in range(QC)] + [
        (kt, wk, pk_d, 1, i, "pkc") for i in range(QC)
    ]
    ft1_iter = iter(ft1_list)

    # ---- attention per head pair p (heads 2p at part 0:64, 2p+1 at 64:128) ----
    def attention_pair(p, interleave):
        step = 0
        for qc in range(QC):
            pv0 = ps_mm.tile([128, 512], F32, tag="mm")
            pv1 = ps_mm.tile([128, 512], F32, tag="mm")
            for k in range(TT):
                sc = ps_sc.tile([128, 1024], F32, tag="sc")
                nc.tensor.matmul(
                    sc[:, 0:512],
                    kt[0:64, p, k * 128 : (k + 1) * 128],
                    qt[0:64, p, qc * 512 : (qc + 1) * 512],
                    start=True,
                    stop=True,
                )
                nc.tensor.matmul(
                    sc[:, 512:1024],
                    kt[64:128, p, k * 128 : (k + 1) * 128],
                    qt[64:128, p, qc * 512 : (qc + 1) * 512],
                    start=True,
                    stop=True,
                )
                ex = expp.tile([128, 1024], F32R, tag="ex")
                nc.scalar.activation(ex[:], sc[:], AF.Exp, scale=float(SCALE))
                nc.tensor.matmul(
                    pv0[0:65, :],
                    vaug[:, k, 2 * p, :],
                    ex[:, 0:512],
                    start=(k == 0),
                    stop=(k == TT - 1),
                )
                nc.tensor.matmul(
                    pv1[0:65, :],
                    vaug[:, k, 2 * p + 1, :],
                    ex[:, 512:1024],
                    start=(k == 0),
                    stop=(k == TT - 1),
                )
                if interleave and step % 8 == 7:
                    args = next(ft1_iter, None)
                    if args is not None:
                        qkt_chunk(*args)
                step += 1
            # normalize: rows 0:64 / row 64 (denominator)
            for h, pv in ((2 * p, pv0), (2 * p + 1, pv1)):
                pvs = pvsp.tile([65, 512], F32, tag="pvs")
                nc.vector.tensor_copy(pvs[:], pv[0:65, :])
                rc = rcp.tile([65, 512], F32R, tag="rc")
                with nc.allow_low_precision(reason="f32r for matmul rhs"):
                    nc.vector.reciprocal(rc[64:65, :], pvs[64:65, :])
                bc = ps_sc.tile([128, 1024], F32, tag="sc")
                nc.tensor.matmul(
                    bc[0:64, 0:512],
                    ones_bc[64:65, :],
                    rc[64:65, :],
                    start=True,
                    stop=True,
                )
                if h % 2 == 0:
                    nc.vector.tensor_tensor(
                        apn[0:64, p, qc * 512 : (qc + 1) * 512],
                        pvs[0:64, :],
                        bc[0:64, 0:512],
                        ALU.mult,
                    )
                else:
                    st = stp.tile([64, 512], F32R, tag="st")
                    nc.vector.tensor_tensor(
                        st[:], pvs[0:64, :], bc[0:64, 0:512], ALU.mult
                    )
                    nc.sync.dma_start(
                        apn[64:128, p, qc * 512 : (qc + 1) * 512], st[:]
                    )

    attention_pair(0, interleave=True)
    for args in ft1_iter:
        qkt_chunk(*args)
    attention_pair(1, interleave=False)

    # ---- out-projection: y^T[m, t] = sum_ft WoT[ft].T @ apn[ft] ----
    for mt in range(DT):
        for tcc in range(QC):
            ps = ps_mm.tile([128, 512], F32, tag="mm")
            for ft in range(2):
                nc.tensor.matmul(
                    ps[:],
                    wo[:, ft, mt * 128 : (mt + 1) * 128],
                    apn[:, ft, tcc * 512 : (tcc + 1) * 512],
                    start=(ft == 0),
                    stop=(ft == 1),
                )
            yst = ystp.tile([128, 512], F32, tag="yst")
            nc.scalar.activation(yst[:], ps[:], AF.Copy, bias=0.0)
            nc.sync.dma_start(
                yt_d[mt * 128 : (mt + 1) * 128, tcc * 512 : (tcc + 1) * 512],
                yst[:],
            )


_NC = {}


def _get_nc(iters=1):
    if iters not in _NC:
        _NC[iters] = build_nc(iters)
    return _NC[iters]


def _sinusoid_pe():
    pos = np.arange(MAX_SEQ_LEN, dtype=np.float32)[:, None]
    div = np.exp(
        np.arange(0, D, 2, dtype=np.float32) * np.float32(-np.log(10000.0) / D)
    )
    ang = pos * div[None, :]
    pe = np.stack([np.sin(ang), np.cos(ang)], axis=-1).reshape(MAX_SEQ_LEN, D)
    return pe.astype(np.float32)


def make_in_maps(x, rel_emb, alpha, Wq, bq, Wk, bk, Wv, bv, Wo, bo):
    alpha = np.float32(alpha)
    abs_pe = _sinusoid_pe()[:S]
    rel_pe = rel_emb[MAX_SEQ_LEN - S : MAX_SEQ_LEN]
    pe = (alpha * abs_pe + (np.float32(1.0) - alpha) * rel_pe).astype(np.float32)

    in_maps = []
    for c in range(NCORES):
        b, g = divmod(c, 4)
        fsl = slice(g * F, (g + 1) * F)
        wq_s, wk_s, wv_s = Wq[fsl], Wk[fsl], Wv[fsl]
        in_maps.append(
            {
                "xt": np.ascontiguousarray(x[b].T),
                "wqt": np.ascontiguousarray(wq_s.T),
                "wkt": np.ascontiguousarray(wk_s.T),
                "wvt": np.ascontiguousarray(wv_s.T),
                "wot": np.ascontiguousarray(Wo[:, fsl].T),
                "pq": np.ascontiguousarray((pe @ wq_s.T + bq[fsl]).T),
                "pk": np.ascontiguousarray((pe @ wk_s.T + bk[fsl]).T),
                "pv": np.ascontiguousarray(pe @ wv_s.T + bv[fsl]),
            }
        )
    return in_maps


def unshard(results, bo):
    y = np.empty((B, S, D), dtype=np.float32)
    for b in range(B):
        acc = results[4 * b]["yt"].astype(np.float32).copy()
        for g in range(1, 4):
            acc += results[4 * b + g]["yt"]
        y[b] = acc.T + bo
    return y


def kernel(x, rel_emb, alpha, Wq, bq, Wk, bk, Wv, bv, Wo, bo, **kw):
    x = np.asarray(x, dtype=np.float32)
    args = [
        np.asarray(a, dtype=np.float32)
        for a in (rel_emb, alpha, Wq, bq, Wk, bk, Wv, bv, Wo, bo)
    ]
    nc = _get_nc()
    in_maps = make_in_maps(x, *args)
    res = run_bass_kernel_spmd(nc, in_maps, core_ids=list(range(NCORES)))
    return unshard(res.results, args[-1])
